# revision 53
# baseline (speedup 1.0000x reference)
"""Trainium2 Bass kernel for nn_Attention_14663018349107 (v2).

Reference computation (B=1, T=4096, D=512, H=8, hd=64, CTX_IN=384):
    Q  = query @ q_w.T + q_b                                  (T, D)
    kv = Conv1d(context^T, kv_w, stride=2) + kv_b             (2D, T) channel-major
    KV = raw-view of kv as (T, 2, D)  [torch .view scrambling]
    K  = KV[:,0] + pos ; V = KV[:,1] + pos
    out = softmax(Q K^T / 8) V  per head, then @ out_w.T + out_b

Sharding: one head per NeuronCore (8 heads / 8 cores).

Changes over the 283 us/core v1 baseline (measured ~172 us/core, rel err
1.08e-2 vs the 2e-2 gate):
  - The whole matmul pipeline runs in fp8e4: conv / Q-proj use DoubleRow
    (contraction pairs, stride%16 ISA rule -> V rows padded to 80), scores
    run 2-way row-packed fp8, attn@V runs as one DR matmul per 256 keys.
    Host quantizes query/weights/context to e4m3; K^T/Q^T/V/exp(S) are
    quantized on device.  conv-K / Q-proj compute only the lower 64
    partitions; the duplicated upper half is made by on-chip DMA.
  - exp(S) alternates waves between the Scalar engine (true exp, fp8 out)
    and the Vector engine (Schraudolph: int8(s/ln2+55.75) bitcast as e4m3
    ~= exp(s/8), ~6% weight noise that washes out over the ~3.5k-key
    softmax support).
  - attn@V is software-pipelined TWO waves behind its scores: the PE
    executes matmuls in program order, so this keeps the PE busy with
    scores while exp runs on ACT/DVE.  This also keeps the PE stream
    dense enough that the HAM governor holds K=8/8 (2.4 GHz) instead of
    oscillating to half clock -- worth ~55 us alone.
  - softmax normalization happens in the host gather: the ones-column
    denominator row of the attention output ships per q-chunk (den_p) and
    the host scales each head's partial output by 1/den before summing.
    The zero row of ow kills the denominator row in the out-projection.
  - All Q-projections run up front (covers the conv-weight DMA), outputs
    ship as bf16 partials.
"""

import math

import numpy as np
import ml_dtypes

SEQ = 4096
DIM = 512
HEADS = 8
HD = 64
CTX_IN = 384
N_CORES = 8

_CACHE = {}

# Schraudolph exp in e4m3 bits: exp(s/8) ~ bitcast_e4m3(int8(s/ln2 + BETA)).
# BETA = 56 - C with C=0.25 splitting the round-vs-trunc convert ambiguity.
ALPHA = 1.0 / math.log(2.0)
BETA = 55.75

# jg indices (16 per q-chunk, 256 keys each) handled by the Vector engine
# via Schraudolph; the rest go to the Scalar engine's real exp.
DVE_JGS = frozenset({1, 3, 5, 7, 9, 11, 13})


def _build_program():
    """Build (and cache) the single-core SPMD Bass program."""
    if "nc" in _CACHE:
        return _CACHE["nc"]

    from contextlib import ExitStack

    import concourse.bacc as bacc
    import concourse.mybir as mybir
    import concourse.tile as tile

    f32 = mybir.dt.float32
    f32r = mybir.dt.float32r
    bf16 = mybir.dt.bfloat16
    fp8 = mybir.dt.float8e4
    i8 = mybir.dt.int8
    EXP = mybir.ActivationFunctionType.Exp
    DR = mybir.MatmulPerfMode.DoubleRow
    MUL = mybir.AluOpType.mult
    ADD = mybir.AluOpType.add

    nc = bacc.Bacc("TRN2", target_bir_lowering=False, debug=False, num_devices=N_CORES)

    # ---- DRAM I/O (per-core content, host pre-laid-out) ----
    qry = nc.dram_tensor("qry_t", [8, 128, 4, 512], fp8, kind="ExternalInput").ap()
    qw = nc.dram_tensor("qw_t", [128, 4, 64], fp8, kind="ExternalInput").ap()
    w2 = nc.dram_tensor("w2_t", [128, 6, 1024], fp8, kind="ExternalInput").ap()
    ckt = nc.dram_tensor("ckt", [4, 128, 6, 64], fp8, kind="ExternalInput").ap()
    cvt = nc.dram_tensor("cvt", [128, 6, 256], fp8, kind="ExternalInput").ap()
    pk = nc.dram_tensor("pos_k", [64, 4096], bf16, kind="ExternalInput").ap()
    pv = nc.dram_tensor("pos_v", [128, 32, 64], bf16, kind="ExternalInput").ap()
    ow = nc.dram_tensor("ow_t", [65, 512], bf16, kind="ExternalInput").ap()
    outp = nc.dram_tensor("out_p", [4096, 512], bf16, kind="ExternalOutput").ap()
    denp = nc.dram_tensor("den_p", [8, 512], bf16, kind="ExternalOutput").ap()

    with tile.TileContext(nc) as tc, ExitStack() as ctx:
        const = ctx.enter_context(tc.tile_pool(name="const", bufs=1))

        # Constant / persistent SBUF tensors (DMA order ~= need order)
        # qw first — the Q-projection phase starts as soon as qw + the first
        # query chunk land; the conv weights stream in behind them.
        qw_sb = const.tile([128, 4, 64], fp8)
        nc.sync.dma_start(qw_sb[:], qw)
        w2_sb = const.tile([128, 6, 1024], fp8)
        ckt_sb = const.tile([128, 4, 6, 64], fp8)
        cvt_sb = const.tile([128, 6, 256], fp8)
        pk_sb = const.tile([64, 4096], bf16)
        pv_sb = const.tile([128, 32, 64], bf16)
        ow_r = const.tile([65, 512], bf16)   # row 0 is zero (host-padded)

        kt2_sb = const.tile([128, 4096], fp8)   # K^T duplicated rows 0-63/64-127
        # V (+ ones col), 32 j-chunks; row stride padded to 80 (the DoubleRow
        # ldweights ISA check requires the k-tile-pair stride % 16 == 0)
        v_sb = const.tile([128, 32, 80], fp8)
        qt2_sb = const.tile([128, 4096], fp8)   # Q^T duplicated

        ones128_f32 = const.tile([128, 1], f32)
        nc.vector.memset(ones128_f32[:], 1.0)

        stream1 = ctx.enter_context(tc.tile_pool(name="stream1", bufs=2))

        conv_psum = tc.alloc_tile_pool(name="conv_psum", bufs=2, space="PSUM")

        # ---------------- Q projection, all chunks up front ----------------
        # (fills the PE while the conv weights/context stream in)
        for qc in range(8):
            qry_t = stream1.tile([128, 4, 512], fp8, tag="qry")
            nc.sync.dma_start(qry_t[:], qry[qc])
            q_ps = conv_psum.tile([64, 512], f32, tag="q")
            for m in range(2):
                nc.tensor.matmul(
                    q_ps[:], qw_sb[:, 2 * m:2 * m + 2, :], qry_t[:, 2 * m:2 * m + 2, :],
                    start=(m == 0), stop=(m == 1), perf_mode=DR,
                )
            nc.vector.tensor_copy(qt2_sb[0:64, qc * 512:(qc + 1) * 512], q_ps[:])
        # duplicate Q^T onto the upper partition half (on-chip DMA)
        nc.sync.dma_start(qt2_sb[64:128, :], qt2_sb[0:64, :])
        # weight/pos streams issue behind the whole query stream (the Q
        # projection phase covers their transfer; issuing them earlier
        # starves the qry chunks and stalls the PE at startup)
        nc.sync.dma_start(w2_sb[:], w2)
        for k in range(4):
            nc.sync.dma_start(ckt_sb[:, k], ckt[k])
        nc.sync.dma_start(cvt_sb[:], cvt)
        nc.sync.dma_start(pk_sb[:], pk)
        nc.sync.dma_start(pv_sb[:], pv)
        nc.sync.dma_start(ow_r[:], ow)

        # ---------------- conv -> K^T (lower half; upper half via DMA dup) ----
        for k in range(4):
            ck_ps = conv_psum.tile([64, 1024], f32, tag="ck")
            for ch in range(2):
                csl = slice(ch * 512, (ch + 1) * 512)
                for m in range(3):
                    nc.tensor.matmul(
                        ck_ps[:, csl], ckt_sb[:, k, 2 * m:2 * m + 2, :],
                        w2_sb[:, 2 * m:2 * m + 2, csl],
                        start=(m == 0), stop=(m == 2), perf_mode=DR,
                    )
            nc.vector.tensor_add(
                kt2_sb[0:64, 1024 * k:1024 * (k + 1)], ck_ps[:],
                pk_sb[:, 1024 * k:1024 * (k + 1)],
            )
        nc.sync.dma_start(kt2_sb[64:128, :], kt2_sb[0:64, :])

        # ---------------- conv -> V natural (fp8 out) ----------------
        for cc in range(8):
            cv_ps = conv_psum.tile([128, 4, 64], f32, tag="cv")
            for m in range(3):
                nc.tensor.matmul(
                    cv_ps[:], w2_sb[:, 2 * m:2 * m + 2, cc * 128:(cc + 1) * 128],
                    cvt_sb[:, 2 * m:2 * m + 2, :],
                    start=(m == 0), stop=(m == 2), perf_mode=DR,
                )
            # one add covers the 4 j-chunks jc = 8k+cc (stride-8 in dim 1)
            nc.vector.tensor_add(
                v_sb[:, cc:32:8, 1:65], cv_ps[:], pv_sb[:, cc:32:8, :],
            )
        nc.vector.tensor_copy(
            v_sb[:, :, 0:1], ones128_f32[:, None, :].to_broadcast([128, 32, 1])
        )
        conv_psum.release()

        # ---------------- attention (Q-proj fused per q-chunk) ----------------
        psum2 = ctx.enter_context(tc.tile_pool(name="psum2", bufs=2, space="PSUM"))
        ptp = ctx.enter_context(tc.tile_pool(name="ptp", bufs=4))
        otp = ctx.enter_context(tc.tile_pool(name="otp", bufs=2))
        outs = ctx.enter_context(tc.tile_pool(name="outs", bufs=3))

        def emit_outproj(bqc, bot_sb, sq):
            # bufs=1: consecutive outprojs are 3 waves apart, so the single
            # bank recycles without stalling; frees banks for st bufs=3.
            op_ps = psum2.tile([128, 512], f32, tag="op", bufs=1)
            nc.tensor.matmul(
                op_ps[:], bot_sb[:, sq * 128:(sq + 1) * 128], ow_r[:],
                start=True, stop=True,
            )
            out_t = outs.tile([128, 512], bf16, tag="out")
            if sq % 2 == 0:
                nc.vector.tensor_copy(out_t[:], op_ps[:])
            else:
                nc.scalar.copy(out_t[:], op_ps[:])
            r0 = (bqc * 4 + sq) * 128
            nc.sync.dma_start(outp[r0:r0 + 128, :], out_t[:])

        # Out-projection of chunk N is emitted a few score groups into chunk
        # N+1 so the PE fills the DMA/sem latency with score matmuls.
        pending = None
        for qc in range(8):
            qsl = slice(qc * 512, (qc + 1) * 512)
            ot_ps = psum2.tile([65, 512], f32, tag="ot", bufs=1)
            # attn@V for wave jg is emitted AFTER the scores of wave jg+2:
            # the PE executes matmuls in program order, so emitting attn@V
            # right after its own scores would stall the PE for the whole
            # exp latency every wave.  Two waves of software pipelining give
            # the ACT/DVE exp ~2 wave periods of latency budget.
            avq = []
            for jg in range(16):
                # bufs=3: with only 2, scores(jg+2) reuses st(jg)'s bank and
                # stalls on the slower DVE-wave exp reads (~1.65us latency vs
                # the ~1.8us two-period budget leaves no margin).
                st_ps = psum2.tile([128, 1024], f32, tag="st", bufs=3)
                jA, jB = 2 * jg, 2 * jg + 1
                nc.tensor.matmul(
                    st_ps[:, 0:512],
                    kt2_sb[0:64, jA * 128:(jA + 1) * 128], qt2_sb[0:64, qsl],
                    start=True, stop=True, tile_position=(0, 0),
                )
                nc.tensor.matmul(
                    st_ps[:, 512:1024],
                    kt2_sb[64:128, jB * 128:(jB + 1) * 128], qt2_sb[64:128, qsl],
                    start=True, stop=True, tile_position=(64, 0),
                )
                if len(avq) == 2:
                    pjA, ppt = avq.pop(0)
                    nc.tensor.matmul(
                        ot_ps[:], v_sb[:, pjA:pjA + 2, 0:65], ppt[:],
                        start=(pjA == 0), stop=False, perf_mode=DR,
                    )
                pt_t = ptp.tile([128, 2, 512], fp8, tag="pt")
                if jg in DVE_JGS:
                    nc.vector.tensor_scalar(
                        pt_t[:].bitcast(i8), st_ps[:], ALPHA, BETA, MUL, ADD
                    )
                else:
                    nc.scalar.activation(pt_t[:], st_ps[:], EXP, scale=0.125)
                avq.append((jA, pt_t))
                if jg in (5, 8, 11, 14) and pending is not None:
                    emit_outproj(*pending, sq=(jg - 5) // 3)
                    if jg == 14:
                        pending = None
            for pjA, ppt in avq:
                nc.tensor.matmul(
                    ot_ps[:], v_sb[:, pjA:pjA + 2, 0:65], ppt[:],
                    start=False, stop=(pjA == 30), perf_mode=DR,
                )

            # stage attention-out (+denominator row) of this chunk; the
            # out-projection itself is deferred into the next chunk.
            ot_sb = otp.tile([65, 512], bf16, tag="ot_sb")
            nc.scalar.copy(ot_sb[:], ot_ps[:])
            nc.sync.dma_start(denp[qc:qc + 1, :], ot_sb[0:1, :])
            pending = (qc, ot_sb)
        for sq in range(4):
            emit_outproj(*pending, sq=sq)

    nc.compile()
    _CACHE["nc"] = nc
    return nc


def _host_prep(query, context, pos, q_w, q_b, kv_w, kv_b, out_w, out_b):
    """Shard + re-lay-out full inputs into per-core input maps."""
    bf = ml_dtypes.bfloat16
    f8 = ml_dtypes.float8_e4m3  # IEEE e4m3 (bias 7, max 240) == TRN fp8_exp4
    query = np.ascontiguousarray(np.asarray(query, dtype=np.float32)[0])   # (4096, 512)
    ctx2 = np.ascontiguousarray(np.asarray(context, dtype=np.float32)[0])  # (8192, 384)
    pos = np.asarray(pos, dtype=np.float32)                                # (4096, 512)
    q_w = np.asarray(q_w, dtype=np.float32)
    q_b = np.asarray(q_b, dtype=np.float32)
    kv_w = np.asarray(kv_w, dtype=np.float32)
    kv_b = np.asarray(kv_b, dtype=np.float32)
    out_w = np.asarray(out_w, dtype=np.float32)

    assert not np.any(q_b), "kernel build assumes q_b == 0 (true for this problem)"

    # shared tensors
    qry_t = np.ascontiguousarray(
        query.reshape(8, 512, 4, 128).transpose(0, 3, 2, 1)
    ).astype(f8)  # (8, 128, 4, 512): [qc, p, o, q] = query[qc*512+q, o*128+p]
    W2 = np.concatenate([kv_w[:, :, 0], kv_w[:, :, 1]], axis=1)  # (1024, 768)
    w2_t = np.ascontiguousarray(
        W2.T.reshape(6, 128, 1024).transpose(1, 0, 2)
    ).astype(f8)  # (128, 6, 1024): [p, o, c] = W2[c, o*128+p]

    # permutation j = k*1024 + c  <->  t' = 4c + k
    j = np.arange(4096)
    kk, cc = j // 1024, j % 1024
    tprime = 4 * cc + kk

    in_maps = []
    for h in range(HEADS):
        qw_t = np.ascontiguousarray(
            q_w[h * 64:(h + 1) * 64, :].reshape(64, 4, 128).transpose(2, 1, 0)
        ).astype(f8)  # (128, 4, 64): [p, o, d] = q_w[64h+d, o*128+p]

        ckt = np.empty((4, 128, 6, 64), dtype=np.float32)
        cvt_parts = []
        for k in range(4):
            blkK = ctx2[2048 * k + 128 * h: 2048 * k + 128 * h + 128]
            blkV = ctx2[2048 * k + 1024 + 128 * h: 2048 * k + 1024 + 128 * h + 128]
            ckt[k] = blkK.reshape(64, 6, 128).transpose(2, 1, 0)
            cvt_parts.append(blkV.reshape(64, 6, 128).transpose(2, 1, 0))
        ckt = ckt.astype(f8)
        cvt = np.ascontiguousarray(np.concatenate(cvt_parts, axis=2)).astype(f8)  # (128, 6, 256)

        pos_h = pos[tprime, h * 64:(h + 1) * 64]  # (4096, 64) permuted rows
        bias_c = kv_b[cc]                          # (4096,) = kv_b[c(j)]
        pos_k = np.ascontiguousarray(pos_h.T + bias_c[None, :]).astype(bf)  # (64, 4096)
        pos_v = np.ascontiguousarray(
            (pos_h + bias_c[:, None]).reshape(32, 128, 64).transpose(1, 0, 2)
        ).astype(bf)  # (128, 32, 64)

        ow_t = np.zeros((65, 512), dtype=np.float32)  # row 0 zero (kills denom row)
        ow_t[1:65] = out_w[:, h * 64:(h + 1) * 64].T
        ow_t = ow_t.astype(bf)

        in_maps.append({
            "qry_t": qry_t,
            "qw_t": qw_t,
            "w2_t": w2_t,
            "ckt": ckt,
            "cvt": cvt,
            "pos_k": pos_k,
            "pos_v": pos_v,
            "ow_t": ow_t,
        })
    return in_maps


def kernel(query, context, pos, q_w, q_b, kv_w, kv_b, out_w, out_b):
    """Full-input, full-output entry point. Runs SPMD on NeuronCores 0-7."""
    from concourse.bass_utils import run_bass_kernel_spmd

    nc = _build_program()
    in_maps = _host_prep(query, context, pos, q_w, q_b, kv_w, kv_b, out_w, out_b)

    res = run_bass_kernel_spmd(nc, in_maps, core_ids=list(range(N_CORES)))

    out = np.zeros((4096, 512), dtype=np.float64)
    for r in res.results:
        den = np.asarray(r["den_p"]).astype(np.float64).reshape(4096)
        out += np.asarray(r["out_p"]).astype(np.float64) / den[:, None]
    out += np.asarray(out_b, dtype=np.float64)[None, :]
    return out[None].astype(np.float32)


# revision 58
# speedup vs baseline: 1.1247x; 1.1247x over previous
"""Trainium2 Bass kernel for nn_Attention_14663018349107 (v2).

Reference computation (B=1, T=4096, D=512, H=8, hd=64, CTX_IN=384):
    Q  = query @ q_w.T + q_b                                  (T, D)
    kv = Conv1d(context^T, kv_w, stride=2) + kv_b             (2D, T) channel-major
    KV = raw-view of kv as (T, 2, D)  [torch .view scrambling]
    K  = KV[:,0] + pos ; V = KV[:,1] + pos
    out = softmax(Q K^T / 8) V  per head, then @ out_w.T + out_b

Sharding: one head per NeuronCore (8 heads / 8 cores).

Changes over the 283 us/core v1 baseline (measured ~172 us/core, rel err
1.08e-2 vs the 2e-2 gate):
  - The whole matmul pipeline runs in fp8e4: conv / Q-proj use DoubleRow
    (contraction pairs, stride%16 ISA rule -> V rows padded to 80), scores
    run 2-way row-packed fp8, attn@V runs as one DR matmul per 256 keys.
    Host quantizes query/weights/context to e4m3; K^T/Q^T/V/exp(S) are
    quantized on device.  conv-K / Q-proj compute only the lower 64
    partitions; the duplicated upper half is made by on-chip DMA.
  - exp(S) alternates waves between the Scalar engine (true exp, fp8 out)
    and the Vector engine (Schraudolph: int8(s/ln2+55.75) bitcast as e4m3
    ~= exp(s/8), ~6% weight noise that washes out over the ~3.5k-key
    softmax support).
  - attn@V is software-pipelined TWO waves behind its scores: the PE
    executes matmuls in program order, so this keeps the PE busy with
    scores while exp runs on ACT/DVE.  This also keeps the PE stream
    dense enough that the HAM governor holds K=8/8 (2.4 GHz) instead of
    oscillating to half clock -- worth ~55 us alone.
  - softmax normalization happens in the host gather: the ones-column
    denominator row of the attention output ships per q-chunk (den_p) and
    the host scales each head's partial output by 1/den before summing.
    The zero row of ow kills the denominator row in the out-projection.
  - All Q-projections run up front (covers the conv-weight DMA), outputs
    ship as bf16 partials.
"""

import math

import numpy as np
import ml_dtypes

SEQ = 4096
DIM = 512
HEADS = 8
HD = 64
CTX_IN = 384
N_CORES = 8

_CACHE = {}

# Schraudolph exp in e4m3 bits: exp(s/8) ~ bitcast_e4m3(int8(s/ln2 + BETA)).
# BETA = 56 - C with C=0.25 splitting the round-vs-trunc convert ambiguity.
ALPHA = 1.0 / math.log(2.0)
BETA = 55.75

# jg indices (16 per q-chunk, 256 keys each) handled by the Vector engine
# via Schraudolph; the rest go to the Scalar engine's real exp.
DVE_JGS = frozenset({1, 3, 5, 7, 9, 11, 13})


def _build_program():
    """Build (and cache) the single-core SPMD Bass program."""
    if "nc" in _CACHE:
        return _CACHE["nc"]

    from contextlib import ExitStack

    import concourse.bacc as bacc
    import concourse.mybir as mybir
    import concourse.tile as tile

    f32 = mybir.dt.float32
    f32r = mybir.dt.float32r
    bf16 = mybir.dt.bfloat16
    fp8 = mybir.dt.float8e4
    i8 = mybir.dt.int8
    EXP = mybir.ActivationFunctionType.Exp
    DR = mybir.MatmulPerfMode.DoubleRow
    MUL = mybir.AluOpType.mult
    ADD = mybir.AluOpType.add

    nc = bacc.Bacc("TRN2", target_bir_lowering=False, debug=False, num_devices=N_CORES)

    # ---- DRAM I/O (per-core content, host pre-laid-out) ----
    qry = nc.dram_tensor("qry_t", [8, 128, 4, 512], fp8, kind="ExternalInput").ap()
    qw = nc.dram_tensor("qw_t", [128, 4, 64], fp8, kind="ExternalInput").ap()
    w2 = nc.dram_tensor("w2_t", [128, 6, 1024], fp8, kind="ExternalInput").ap()
    ckt = nc.dram_tensor("ckt", [4, 128, 6, 64], fp8, kind="ExternalInput").ap()
    cvt = nc.dram_tensor("cvt", [128, 6, 256], fp8, kind="ExternalInput").ap()
    pk = nc.dram_tensor("pos_k", [64, 4096], bf16, kind="ExternalInput").ap()
    pv = nc.dram_tensor("pos_v", [128, 32, 64], bf16, kind="ExternalInput").ap()
    ow = nc.dram_tensor("ow_t", [65, 512], bf16, kind="ExternalInput").ap()
    outp = nc.dram_tensor("out_p", [4096, 512], bf16, kind="ExternalOutput").ap()
    denp = nc.dram_tensor("den_p", [8, 512], bf16, kind="ExternalOutput").ap()

    with tile.TileContext(nc) as tc, ExitStack() as ctx:
        const = ctx.enter_context(tc.tile_pool(name="const", bufs=1))

        # Constant / persistent SBUF tensors (DMA order ~= need order)
        # qw first — the Q-projection phase starts as soon as qw + the first
        # query chunk land; the conv weights stream in behind them.
        qw_sb = const.tile([128, 4, 64], fp8)
        nc.sync.dma_start(qw_sb[:], qw)
        w2_sb = const.tile([128, 6, 1024], fp8)
        ckt_sb = const.tile([128, 4, 6, 64], fp8)
        cvt_sb = const.tile([128, 6, 256], fp8)
        pk_sb = const.tile([64, 4096], bf16)
        pv_sb = const.tile([128, 32, 64], bf16)
        ow_r = const.tile([65, 512], bf16)   # row 0 is zero (host-padded)

        kt2_sb = const.tile([128, 4096], fp8)   # K^T duplicated rows 0-63/64-127
        # V (+ ones col), 32 j-chunks; row stride padded to 80 (the DoubleRow
        # ldweights ISA check requires the k-tile-pair stride % 16 == 0)
        v_sb = const.tile([128, 32, 80], fp8)
        qt2_sb = const.tile([128, 4096], fp8)   # Q^T duplicated

        ones128_f32 = const.tile([128, 1], f32)
        nc.vector.memset(ones128_f32[:], 1.0)

        stream1 = ctx.enter_context(tc.tile_pool(name="stream1", bufs=2))

        conv_psum = tc.alloc_tile_pool(name="conv_psum", bufs=2, space="PSUM")

        # ---------------- Q projection, all chunks up front ----------------
        # (fills the PE while the conv weights/context stream in)
        for qc in range(8):
            qry_t = stream1.tile([128, 4, 512], fp8, tag="qry")
            nc.sync.dma_start(qry_t[:], qry[qc])
            q_ps = conv_psum.tile([64, 512], f32, tag="q")
            for m in range(2):
                nc.tensor.matmul(
                    q_ps[:], qw_sb[:, 2 * m:2 * m + 2, :], qry_t[:, 2 * m:2 * m + 2, :],
                    start=(m == 0), stop=(m == 1), perf_mode=DR,
                )
            nc.vector.tensor_copy(qt2_sb[0:64, qc * 512:(qc + 1) * 512], q_ps[:])
            if qc == 0:
                # weight/pos streams issue behind the first query chunk
                nc.sync.dma_start(w2_sb[:], w2)
                for k in range(4):
                    nc.sync.dma_start(ckt_sb[:, k], ckt[k])
                nc.sync.dma_start(cvt_sb[:], cvt)
                nc.sync.dma_start(pk_sb[:], pk)
                nc.sync.dma_start(pv_sb[:], pv)
                nc.sync.dma_start(ow_r[:], ow)
        # duplicate Q^T onto the upper partition half (on-chip DMA)
        nc.sync.dma_start(qt2_sb[64:128, :], qt2_sb[0:64, :])

        # ---------------- conv -> K^T (lower half; upper half via DMA dup) ----
        for k in range(4):
            ck_ps = conv_psum.tile([64, 1024], f32, tag="ck")
            for ch in range(2):
                csl = slice(ch * 512, (ch + 1) * 512)
                for m in range(3):
                    nc.tensor.matmul(
                        ck_ps[:, csl], ckt_sb[:, k, 2 * m:2 * m + 2, :],
                        w2_sb[:, 2 * m:2 * m + 2, csl],
                        start=(m == 0), stop=(m == 2), perf_mode=DR,
                    )
            nc.vector.tensor_add(
                kt2_sb[0:64, 1024 * k:1024 * (k + 1)], ck_ps[:],
                pk_sb[:, 1024 * k:1024 * (k + 1)],
            )
        nc.sync.dma_start(kt2_sb[64:128, :], kt2_sb[0:64, :])

        # ---------------- conv -> V natural (fp8 out) ----------------
        nc.vector.tensor_copy(
            v_sb[:, :, 0:1], ones128_f32[:, None, :].to_broadcast([128, 32, 1])
        )
        for cc in range(8):
            cv_ps = conv_psum.tile([128, 4, 64], f32, tag="cv")
            for m in range(3):
                nc.tensor.matmul(
                    cv_ps[:], w2_sb[:, 2 * m:2 * m + 2, cc * 128:(cc + 1) * 128],
                    cvt_sb[:, 2 * m:2 * m + 2, :],
                    start=(m == 0), stop=(m == 2), perf_mode=DR,
                )
            # one add covers the 4 j-chunks jc = 8k+cc (stride-8 in dim 1)
            nc.vector.tensor_add(
                v_sb[:, cc:32:8, 1:65], cv_ps[:], pv_sb[:, cc:32:8, :],
            )
        conv_psum.release()

        # ---------------- attention (Q-proj fused per q-chunk) ----------------
        psum2 = ctx.enter_context(tc.tile_pool(name="psum2", bufs=2, space="PSUM"))
        ptp = ctx.enter_context(tc.tile_pool(name="ptp", bufs=4))
        otp = ctx.enter_context(tc.tile_pool(name="otp", bufs=2))
        outs = ctx.enter_context(tc.tile_pool(name="outs", bufs=3))

        def emit_outproj(bqc, bot_sb, sq, tag="op", bufs=1):
            # bufs=1: consecutive outprojs are 3 waves apart, so the single
            # bank recycles without stalling; frees banks for st bufs=3.
            # The final chunk's outprojs instead rotate through the dead
            # score ring (tag "st", bufs=3) so the tail drain isn't
            # serialized on one bank.
            op_ps = psum2.tile([128, 512], f32, tag=tag, bufs=bufs)
            nc.tensor.matmul(
                op_ps[:], bot_sb[:, sq * 128:(sq + 1) * 128], ow_r[:],
                start=True, stop=True,
            )
            out_t = outs.tile([128, 512], bf16, tag="out")
            if sq % 2 == 0:
                nc.vector.tensor_copy(out_t[:], op_ps[:])
            else:
                nc.scalar.copy(out_t[:], op_ps[:])
            r0 = (bqc * 4 + sq) * 128
            nc.sync.dma_start(outp[r0:r0 + 128, :], out_t[:])

        # Out-projection of chunk N is emitted a few score groups into chunk
        # N+1 so the PE fills the DMA/sem latency with score matmuls.
        pending = None
        for qc in range(8):
            qsl = slice(qc * 512, (qc + 1) * 512)
            ot_ps = psum2.tile([65, 512], f32, tag="ot", bufs=1)
            # attn@V for wave jg is emitted AFTER the scores of wave jg+2:
            # the PE executes matmuls in program order, so emitting attn@V
            # right after its own scores would stall the PE for the whole
            # exp latency every wave.  Two waves of software pipelining give
            # the ACT/DVE exp ~2 wave periods of latency budget.
            avq = []
            for jg in range(16):
                # bufs=3: with only 2, scores(jg+2) reuses st(jg)'s bank and
                # stalls on the slower DVE-wave exp reads (~1.65us latency vs
                # the ~1.8us two-period budget leaves no margin).
                st_ps = psum2.tile([128, 1024], f32, tag="st", bufs=3)
                jA, jB = 2 * jg, 2 * jg + 1
                nc.tensor.matmul(
                    st_ps[:, 0:512],
                    kt2_sb[0:64, jA * 128:(jA + 1) * 128], qt2_sb[0:64, qsl],
                    start=True, stop=True, tile_position=(0, 0),
                )
                nc.tensor.matmul(
                    st_ps[:, 512:1024],
                    kt2_sb[64:128, jB * 128:(jB + 1) * 128], qt2_sb[64:128, qsl],
                    start=True, stop=True, tile_position=(64, 0),
                )
                if len(avq) == 2:
                    pjA, ppt = avq.pop(0)
                    nc.tensor.matmul(
                        ot_ps[:], v_sb[:, pjA:pjA + 2, 0:65], ppt[:],
                        start=(pjA == 0), stop=False, perf_mode=DR,
                    )
                pt_t = ptp.tile([128, 2, 512], fp8, tag="pt")
                if jg in DVE_JGS:
                    nc.vector.tensor_scalar(
                        pt_t[:].bitcast(i8), st_ps[:], ALPHA, BETA, MUL, ADD
                    )
                else:
                    nc.scalar.activation(pt_t[:], st_ps[:], EXP, scale=0.125)
                avq.append((jA, pt_t))
                if jg in (5, 8, 11, 14) and pending is not None:
                    emit_outproj(*pending, sq=(jg - 5) // 3)
                    if jg == 14:
                        pending = None
            for pjA, ppt in avq:
                nc.tensor.matmul(
                    ot_ps[:], v_sb[:, pjA:pjA + 2, 0:65], ppt[:],
                    start=False, stop=(pjA == 30), perf_mode=DR,
                )

            # stage attention-out (+denominator row) of this chunk; the
            # out-projection itself is deferred into the next chunk.
            ot_sb = otp.tile([65, 512], bf16, tag="ot_sb")
            nc.scalar.copy(ot_sb[:], ot_ps[:])
            nc.sync.dma_start(denp[qc:qc + 1, :], ot_sb[0:1, :])
            pending = (qc, ot_sb)
        for sq in range(4):
            emit_outproj(*pending, sq=sq, tag="st", bufs=3)

    nc.compile()
    _CACHE["nc"] = nc
    return nc


def _host_prep(query, context, pos, q_w, q_b, kv_w, kv_b, out_w, out_b):
    """Shard + re-lay-out full inputs into per-core input maps."""
    bf = ml_dtypes.bfloat16
    f8 = ml_dtypes.float8_e4m3  # IEEE e4m3 (bias 7, max 240) == TRN fp8_exp4
    query = np.ascontiguousarray(np.asarray(query, dtype=np.float32)[0])   # (4096, 512)
    ctx2 = np.ascontiguousarray(np.asarray(context, dtype=np.float32)[0])  # (8192, 384)
    pos = np.asarray(pos, dtype=np.float32)                                # (4096, 512)
    q_w = np.asarray(q_w, dtype=np.float32)
    q_b = np.asarray(q_b, dtype=np.float32)
    kv_w = np.asarray(kv_w, dtype=np.float32)
    kv_b = np.asarray(kv_b, dtype=np.float32)
    out_w = np.asarray(out_w, dtype=np.float32)

    assert not np.any(q_b), "kernel build assumes q_b == 0 (true for this problem)"

    # shared tensors
    qry_t = np.ascontiguousarray(
        query.reshape(8, 512, 4, 128).transpose(0, 3, 2, 1)
    ).astype(f8)  # (8, 128, 4, 512): [qc, p, o, q] = query[qc*512+q, o*128+p]
    W2 = np.concatenate([kv_w[:, :, 0], kv_w[:, :, 1]], axis=1)  # (1024, 768)
    w2_t = np.ascontiguousarray(
        W2.T.reshape(6, 128, 1024).transpose(1, 0, 2)
    ).astype(f8)  # (128, 6, 1024): [p, o, c] = W2[c, o*128+p]

    # permutation j = k*1024 + c  <->  t' = 4c + k
    j = np.arange(4096)
    kk, cc = j // 1024, j % 1024
    tprime = 4 * cc + kk

    in_maps = []
    for h in range(HEADS):
        qw_t = np.ascontiguousarray(
            q_w[h * 64:(h + 1) * 64, :].reshape(64, 4, 128).transpose(2, 1, 0)
        ).astype(f8)  # (128, 4, 64): [p, o, d] = q_w[64h+d, o*128+p]

        ckt = np.empty((4, 128, 6, 64), dtype=np.float32)
        cvt_parts = []
        for k in range(4):
            blkK = ctx2[2048 * k + 128 * h: 2048 * k + 128 * h + 128]
            blkV = ctx2[2048 * k + 1024 + 128 * h: 2048 * k + 1024 + 128 * h + 128]
            ckt[k] = blkK.reshape(64, 6, 128).transpose(2, 1, 0)
            cvt_parts.append(blkV.reshape(64, 6, 128).transpose(2, 1, 0))
        ckt = ckt.astype(f8)
        cvt = np.ascontiguousarray(np.concatenate(cvt_parts, axis=2)).astype(f8)  # (128, 6, 256)

        pos_h = pos[tprime, h * 64:(h + 1) * 64]  # (4096, 64) permuted rows
        bias_c = kv_b[cc]                          # (4096,) = kv_b[c(j)]
        pos_k = np.ascontiguousarray(pos_h.T + bias_c[None, :]).astype(bf)  # (64, 4096)
        pos_v = np.ascontiguousarray(
            (pos_h + bias_c[:, None]).reshape(32, 128, 64).transpose(1, 0, 2)
        ).astype(bf)  # (128, 32, 64)

        ow_t = np.zeros((65, 512), dtype=np.float32)  # row 0 zero (kills denom row)
        ow_t[1:65] = out_w[:, h * 64:(h + 1) * 64].T
        ow_t = ow_t.astype(bf)

        in_maps.append({
            "qry_t": qry_t,
            "qw_t": qw_t,
            "w2_t": w2_t,
            "ckt": ckt,
            "cvt": cvt,
            "pos_k": pos_k,
            "pos_v": pos_v,
            "ow_t": ow_t,
        })
    return in_maps


def kernel(query, context, pos, q_w, q_b, kv_w, kv_b, out_w, out_b):
    """Full-input, full-output entry point. Runs SPMD on NeuronCores 0-7."""
    from concourse.bass_utils import run_bass_kernel_spmd

    nc = _build_program()
    in_maps = _host_prep(query, context, pos, q_w, q_b, kv_w, kv_b, out_w, out_b)

    res = run_bass_kernel_spmd(nc, in_maps, core_ids=list(range(N_CORES)))

    out = np.zeros((4096, 512), dtype=np.float64)
    for r in res.results:
        den = np.asarray(r["den_p"]).astype(np.float64).reshape(4096)
        out += np.asarray(r["out_p"]).astype(np.float64) / den[:, None]
    out += np.asarray(out_b, dtype=np.float64)[None, :]
    return out[None].astype(np.float32)
